# revision 48
# baseline (speedup 1.0000x reference)
"""Trainium2 Bass kernel for AdditiveAttention (per-batch bmm attention).

Per batch element b (x: (C, N) with C=256, KC=32, N=48*48=2304):
    q = Wq @ x + bq            (KC, N)
    k = Wk @ x + bk            (KC, N)
    v = Wv @ x + bv            (C, N)
    s = (q^T k) / sqrt(KC)     (N, N)
    a = softmax(s, axis=-1)
    out = v @ a^T              (C, N)
    y = gamma * out + x

Distribution: data-parallel over batch B=16 across 8 cores (2 per core);
the small channel-mixing weights are replicated.

Device strategy (v2 — fp8 DoubleRow + PE row-tiling):
  - x is shipped as fp8 "cc-pairs" [128, 2, N] so every producer matmul can
    use fp8 DoubleRow (contraction 256 = 2 k-tiles of 128 at 0.5 cyc/row).
  - q, k are produced REPLICATED 4x across partition quadrants (the Wq/Wk
    stationary is pre-tiled on the host), so the K=32 scores matmuls can be
    issued to distinct 32-row PE bands via tile_position and run
    concurrently (the 128x128 array is 16 independent 32x32 sub-arrays).
  - scoresT[j, i] chunks go through PSUM in 2-j-block groups; exp runs as
    1024-wide activations writing fp8 directly in the DoubleRow pair layout
    [128, 2, 512] that the out-matmul consumes.
  - out is computed in [c, i] layout: out[c,i] = sum_j v[c,j] e[j,i] via
    DoubleRow matmuls with stationary vt-pairs (fp8), slab-major so the
    PSUM accumulates over all 9 j-pairs per 512-i-chunk.
  - The softmax denominator D[i] = sum_j e[j,i] comes from an extra
    all-ones fp8 stationary slab -> D lands broadcast on all 128
    partitions; normalization is then a plain tensor_tensor multiply by
    reciprocal(D), and the residual (x + gamma*bv, folded on host) is a
    second tensor_tensor add.
  - gamma is folded into Wv on the host; gamma=0 stays exact because
    y = 0*rd + xfb = x in f32.
"""

import math
import time
from contextlib import ExitStack

import numpy as np

import concourse.bass as bass
import concourse.bacc as bacc
import concourse.mybir as mybir
import concourse.tile as tile
from concourse.bass_utils import run_bass_kernel_spmd

B, C, KC, H, W = 16, 256, 32, 48, 48
N = H * W            # 2304
NCORES = 8
BPC = B // NCORES    # batch elements per core = 2
P = 128
NB = N // P          # 18 j-blocks
NJP = NB // 2        # 9 j-pairs
CHUNKS = [(0, 512), (512, 512), (1024, 512), (1536, 512), (2048, 256)]

F32 = mybir.dt.float32
BF16 = mybir.dt.bfloat16
F8 = mybir.dt.float8e4
U8 = mybir.dt.uint8
DR = mybir.MatmulPerfMode.DoubleRow
EXP = mybir.ActivationFunctionType.Exp
R32 = 1.0 / math.sqrt(KC)
EXP_SHIFT = -3.0          # e = exp(s/sqrt(KC) - 3); cancels in softmax, keeps e < fp8e4m3 max
# Schraudolph exp constants for fp8e4m3 bit patterns (bias 7, 3 mantissa):
SCH_A = (8.0 / math.log(2.0)) * R32
SCH_B = 7 * 8 + EXP_SHIFT * (8.0 / math.log(2.0)) - 0.463

# exp engine split by group parity: even groups (psum buffer chain A) stay on
# ScalarE, odd groups go to DVE via Schraudolph -> the two serial exp chains
# advance concurrently on two engines.
DVE_EXP_G = frozenset({1, 3, 5, 7})


class _Builder:
    def __init__(self, dve_exp_g=DVE_EXP_G):
        nc = bacc.Bacc()
        self.nc = nc
        self.dve_exp_g = dve_exp_g
        self.xp = nc.dram_tensor("xp", [BPC, P, 2, N], F8, kind="ExternalInput")
        self.xfb = nc.dram_tensor("xfb", [BPC, 2, P, N], F32, kind="ExternalInput")
        self.wq = nc.dram_tensor("wq", [P, 2, P], F8, kind="ExternalInput")
        self.wk = nc.dram_tensor("wk", [P, 2, P], F8, kind="ExternalInput")
        self.wv = nc.dram_tensor("wv", [P, 2, C], F8, kind="ExternalInput")
        self.bqr = nc.dram_tensor("bqr", [P, 1], F32, kind="ExternalInput")
        self.bkr = nc.dram_tensor("bkr", [P, 1], F32, kind="ExternalInput")
        self.y = nc.dram_tensor("y", [BPC, 2, P, N], F32, kind="ExternalOutput")

    def build(self):
        nc = self.nc
        with tile.TileContext(nc) as tc, ExitStack() as ctx:
            self.tc = tc
            const = ctx.enter_context(tc.tile_pool(name="const", bufs=1))
            # PSUM: 4 (scores) + 3 (out D/c0/c1) + 1 (aux) banks
            self.ps_s = ctx.enter_context(tc.tile_pool(name="ps_s", bufs=2, space="PSUM"))
            self.ps_o = ctx.enter_context(tc.tile_pool(name="ps_o", bufs=2, space="PSUM"))
            self.ps_a = ctx.enter_context(tc.tile_pool(name="ps_a", bufs=1, space="PSUM"))
            self.xppool = ctx.enter_context(tc.tile_pool(name="xppool", bufs=2))
            self.qkpool = ctx.enter_context(tc.tile_pool(name="qkpool", bufs=4))
            self.vtpool = ctx.enter_context(tc.tile_pool(name="vtpool", bufs=2 * NJP))
            self.epool = ctx.enter_context(tc.tile_pool(name="epool", bufs=20))
            self.xfpool = ctx.enter_context(tc.tile_pool(name="xfpool", bufs=8))
            self.ypool = ctx.enter_context(tc.tile_pool(name="ypool", bufs=4))
            self.tmppool = ctx.enter_context(tc.tile_pool(name="tmppool", bufs=4))
            self.rdpool = ctx.enter_context(tc.tile_pool(name="rdpool", bufs=3))

            # ---- consts ----
            warm_act = const.tile([1, 2], F32)
            nc.vector.memset(warm_act, 0.0)
            nc.scalar.activation(out=warm_act, in_=warm_act, func=EXP)
            self.ones_sb = const.tile([P, 2, P], F8)
            nc.gpsimd.memset(self.ones_sb, 1.0)
            self.eshift = const.tile([P, 1], F32)
            nc.vector.memset(self.eshift, EXP_SHIFT)
            self.wq_sb = const.tile([P, 2, P], F8)
            nc.sync.dma_start(out=self.wq_sb, in_=self.wq[:])
            self.wk_sb = const.tile([P, 2, P], F8)
            nc.sync.dma_start(out=self.wk_sb, in_=self.wk[:])
            self.wv_sb = const.tile([P, 2, C], F8)
            nc.sync.dma_start(out=self.wv_sb, in_=self.wv[:])
            self.bq_sb = const.tile([P, 1], F32)
            nc.sync.dma_start(out=self.bq_sb, in_=self.bqr[:])
            self.bk_sb = const.tile([P, 1], F32)
            nc.sync.dma_start(out=self.bk_sb, in_=self.bkr[:])

            # two aux PSUM banks; producers fill 256-halves, copies drain 512.
            # q/vt use the low bank (DVE copies), k the high bank (ScalarE).
            self.aux = self.ps_a.tile([P, 1024], F32, tag="a", name="aux_ps")
            self.exp_ctr = 0

            # ---- HAM warmup: dense dummy matmuls (via ps_o so qk can start) ----
            wtile = const.tile([P, 256], BF16)
            nc.vector.memset(wtile, 0.0)
            for wi in range(21):
                ps = self.ps_o.tile([P, 512], F32, tag="o", name="warm_ps")
                nc.tensor.matmul(
                    ps[:, 0:256], lhsT=wtile[:, 0:P], rhs=wtile[:, 0:256],
                    start=True, stop=True,
                )

            # ---- per-core state ----
            self.xs = {}    # b -> xp tile [P, 2, N] F8
            self.q = {}     # b -> q tile [P, N] BF16 (4x replicated on partitions)
            self.k = {}
            self.vtp = {}   # (b, jp) -> [P, 2, C] F8
            self.es = {}    # (b, t, g) -> e tile [P, 2, 512] F8
            self.xf = {}    # (b, t, s) -> xf tile
            self.rd = {}    # (b, t) -> reciprocal-D tile

            self.emit_x_load(0)
            self.emit_x_load(1)
            self.emit_qk(0)

            # startup: scores(b0,t0) interleaved with vt(b0)
            for g in range(NJP):
                self.emit_scores_exp(0, 0, g)
                self.emit_vt(0, 2 * g)
                self.emit_vt(0, 2 * g + 1)

            seq = [(b, t) for b in range(BPC) for t in range(len(CHUNKS))]
            for idx, (b, t) in enumerate(seq):
                nxt = seq[idx + 1] if idx + 1 < len(seq) else None
                self.emit_chunk(b, t, nxt, idx)
                # inject b1 prologue early
                if idx == 0:
                    self.emit_qk(1)
                if idx == 1:
                    for j in range(NB):
                        self.emit_vt(1, j)

        nc.finalize()
        return nc

    # ---------- phases ----------

    def emit_x_load(self, b):
        # split across two queues so neither blocks the (tiny) const loads
        nc = self.nc
        self.xs[b] = self.xppool.tile([P, 2, N], F8, tag="xp", name="x_sb")
        half = N // 2
        nc.gpsimd.dma_start(
            out=self.xs[b][:, :, 0:half], in_=self.xp[b][:, :, 0:half]
        )
        nc.scalar.dma_start(
            out=self.xs[b][:, :, half:N], in_=self.xp[b][:, :, half:N]
        )

    def emit_qk(self, b, borrow=False):
        """q/k in 512-col steps (two 256 DR matmuls + one wide biased copy).

        k goes through a borrowed ps_o bank with copies on ScalarE, q through
        the aux bank with copies on DVE — two independent chains, so the
        startup latency before scores can begin is roughly halved.
        """
        nc = self.nc
        self.q[b] = self.qkpool.tile([P, N], BF16, tag="q", name="q_sb")
        self.k[b] = self.qkpool.tile([P, N], BF16, tag="k", name="k_sb")
        steps = [(512 * h, 512) for h in range(4)] + [(2048, 256)]
        for i0, w in steps:
            for ci, (w_sb, b_sb, dst) in enumerate(
                (
                    (self.wk_sb, self.bk_sb, self.k[b]),
                    (self.wq_sb, self.bq_sb, self.q[b]),
                )
            ):
                ps = self.aux[:, 512 * ci : 512 * ci + 512]
                nc.tensor.matmul(
                    ps[:, 0:w],
                    lhsT=w_sb,
                    rhs=self.xs[b][:, :, i0 : i0 + w],
                    perf_mode=DR,
                    start=True,
                    stop=True,
                )
                if ci == 0:
                    nc.scalar.activation(
                        out=dst[:, i0 : i0 + w], in_=ps[:, 0:w],
                        func=mybir.ActivationFunctionType.Identity, bias=b_sb,
                    )
                else:
                    nc.vector.tensor_scalar_add(dst[:, i0 : i0 + w], ps[:, 0:w], b_sb)

    def emit_vt(self, b, j):
        """vt[j] = (x_j^T) @ WvT in (j-part, c) layout, stored into fp8 pair
        tiles; the psum->fp8 copy drains a full j-pair (512 wide) at once."""
        nc = self.nc
        jp, par = divmod(j, 2)
        if par == 0:
            self.vtp[(b, jp)] = self.vtpool.tile([P, 2, C], F8, tag="vtp", name="vtp")
        ps = self.aux[:, 512 * (jp % 2) + 256 * par : 512 * (jp % 2) + 256 * (par + 1)]
        nc.tensor.matmul(
            ps,
            lhsT=self.xs[b][:, :, j * P : (j + 1) * P],
            rhs=self.wv_sb,
            perf_mode=DR,
            start=True,
            stop=True,
        )
        if par == 1:
            lo = 512 * (jp % 2)
            if jp % 2:
                nc.vector.tensor_scalar_mul(
                    self.vtp[(b, jp)][:, :, :], self.aux[:, lo : lo + 512], 1.0
                )
            else:
                nc.scalar.activation(
                    out=self.vtp[(b, jp)][:, :, :], in_=self.aux[:, lo : lo + 512],
                    func=mybir.ActivationFunctionType.Copy,
                )

    def emit_scores_exp(self, b, t, g):
        """One 2-j-block scores group + its 1024-wide exp -> fp8 e pair tile."""
        nc = self.nc
        i0, w = CHUNKS[t]
        ps = self.ps_s.tile([P, 2, 512], F32, tag="s", name="s_ps")
        band = (g % 2) * 2  # alternate row-band pairs (0,1) / (2,3)
        for r in range(2):
            j = 2 * g + r
            rb = band + r
            nc.tensor.matmul(
                ps[:, r, 0:w],
                lhsT=self.k[b][32 * rb : 32 * (rb + 1), j * P : (j + 1) * P],
                rhs=self.q[b][32 * rb : 32 * (rb + 1), i0 : i0 + w],
                tile_position=(32 * rb, 0),
                start=True,
                stop=True,
            )
        e = self.epool.tile([P, 2, 512], F8, tag="e", name="e_sb")
        self.exp_ctr += 1
        if self.dve_exp_g and (self.exp_ctr % 2 == 0):
            # Schraudolph exp: fp8 bits = round(s*A + B), saturating convert
            # through uint8 clamps the deep-negative tail to +0.
            nc.vector.tensor_scalar(
                out=e[:, :, 0:w].bitcast(U8),
                in0=ps[:, :, 0:w],
                scalar1=SCH_A,
                scalar2=SCH_B,
                op0=mybir.AluOpType.mult,
                op1=mybir.AluOpType.add,
            )
        else:
            nc.scalar.activation(
                out=e[:, :, 0:w], in_=ps[:, :, 0:w], func=EXP,
                scale=R32, bias=self.eshift,
            )
        self.es[(b, t, g)] = e

    def emit_out_mm(self, b, t, mm_i, slabs):
        """One out-burst step: DR matmul mm_i (slab-major) + epilogue hooks.

        slabs holds [D, c0] from the chunk start; c1's psum tile is allocated
        lazily after the D reciprocal so it can reuse D's bank (ps_o bufs=2).
        """
        nc = self.nc
        i0, w = CHUNKS[t]
        kind, jp = divmod(mm_i, NJP)
        e = self.es[(b, t, jp)]
        lhsT = (
            self.ones_sb
            if kind == 0
            else self.vtp[(b, jp)][:, :, (kind - 1) * P : kind * P]
        )
        nc.tensor.matmul(
            slabs[kind][:, 0:w],
            lhsT=lhsT,
            rhs=e[:, :, 0:w],
            perf_mode=DR,
            start=(jp == 0),
            stop=(jp == NJP - 1),
        )
        mm_i += 1
        if mm_i == 9:
            # D slab complete -> reciprocal (fast approx; D is O(1)..O(500))
            rd = self.rdpool.tile([P, 512], F32, tag="rd", name="rd_sb")
            nc.vector.reciprocal_approx_fast(rd[:, 0:w], slabs[0][:, 0:w])
            self.rd[(b, t)] = rd
            slabs.append(self.ps_o.tile([P, 512], F32, tag="o", name="o_ps"))
        elif mm_i in (18, 27):
            s = mm_i // 9 - 2  # 0 or 1
            tmp = self.tmppool.tile([P, 512], F32, tag="tmp", name="tmp_sb")
            nc.vector.tensor_mul(
                tmp[:, 0:w], slabs[s + 1][:, 0:w], self.rd[(b, t)][:, 0:w]
            )
            yt = self.ypool.tile([P, 512], F32, tag="y", name="y_sb")
            add_eng = nc.vector if (b, t) == (BPC - 1, len(CHUNKS) - 1) else nc.gpsimd
            add_eng.tensor_add(
                yt[:, 0:w], tmp[:, 0:w], self.xf[(b, t, s)][:, 0:w]
            )
            nc.sync.dma_start(out=self.y[b, s, :, i0 : i0 + w], in_=yt[:, 0:w])

    def emit_chunk(self, b, t, nxt, idx):
        """Out-burst + epilogue for (b, t), interleaved with scores for nxt.

        Scores groups are emitted in adjacent pairs (bands 0,1 then 2,3) so
        up to 4 small-K matmuls stream concurrently in distinct PE row
        bands; out-burst DR matmuls fill the gaps while ScalarE works
        through the exps.
        """
        nc = self.nc
        i0, w = CHUNKS[t]

        # prefetch xf one chunk ahead so the epilogue add never waits the DMA
        for tb, tt in ((b, t), nxt or (None, None)):
            if tb is None or (tb, tt, 0) in self.xf:
                continue
            ii0, ww = CHUNKS[tt]
            for s in range(2):
                xf = self.xfpool.tile([P, 512], F32, tag="xf", name="xf_sb")
                nc.sync.dma_start(
                    out=xf[:, 0:ww], in_=self.xfb[tb, s, :, ii0 : ii0 + ww]
                )
                self.xf[(tb, tt, s)] = xf

        slabs = [
            self.ps_o.tile([P, 512], F32, tag="o", name="o_ps") for _ in range(2)
        ]

        # interleave: scores pair-bursts of nxt with out-burst of this chunk
        mm_i = 0
        for h in range(5):
            if nxt is not None:
                for g in (2 * h, 2 * h + 1):
                    if g < NJP:
                        self.emit_scores_exp(nxt[0], nxt[1], g)
            n_out = 6 if h < 4 else 3
            for _ in range(n_out):
                self.emit_out_mm(b, t, mm_i, slabs)
                mm_i += 1


def _build_nc(dve_exp_g=DVE_EXP_G):
    return _Builder(dve_exp_g).build()


_CACHE = {}


def kernel(x, Wq, bq, Wk, bk, Wv, bv, gamma):
    x = np.asarray(x, dtype=np.float32)
    Wq = np.asarray(Wq, dtype=np.float32)
    bq = np.asarray(bq, dtype=np.float32)
    Wk = np.asarray(Wk, dtype=np.float32)
    bk = np.asarray(bk, dtype=np.float32)
    Wv = np.asarray(Wv, dtype=np.float32)
    bv = np.asarray(bv, dtype=np.float32)
    gamma = np.asarray(gamma, dtype=np.float32)
    g = float(gamma[0])
    f8 = mybir.dt.np(F8)

    xfull = x.reshape(B, C, N)

    # x cc-pairs: [B, P, 2, N]
    xpair = np.ascontiguousarray(xfull.reshape(B, 2, P, N).transpose(0, 2, 1, 3))
    # residual + gamma*bv folded: [B, 2, P, N]
    xf = xfull.reshape(B, 2, P, N) + (g * bv).reshape(1, 2, P, 1)
    xf = np.ascontiguousarray(xf, dtype=np.float32)

    def rep_pairs(wT):  # (C, K) -> [P, 2, 4*K] replicated along quadrants
        a = np.ascontiguousarray(wT.reshape(2, P, wT.shape[1]).transpose(1, 0, 2))
        return np.tile(a, (1, 1, 4))

    wq_h = rep_pairs(Wq.T).astype(f8)                       # (P, 2, 128)
    wk_h = rep_pairs(Wk.T).astype(f8)
    wv_h = np.ascontiguousarray(
        (Wv * g).T.reshape(2, P, C).transpose(1, 0, 2)
    ).astype(f8)                                            # (P, 2, 256)
    bq_h = np.ascontiguousarray(np.tile(bq, 4).reshape(P, 1), dtype=np.float32)
    bk_h = np.ascontiguousarray(np.tile(bk, 4).reshape(P, 1), dtype=np.float32)

    if "nc" not in _CACHE:
        _CACHE["nc"] = _build_nc()
    nc = _CACHE["nc"]

    in_maps = []
    for core in range(NCORES):
        bsl = slice(core * BPC, (core + 1) * BPC)
        in_maps.append(
            {
                "xp": xpair[bsl].astype(f8),
                "xfb": xf[bsl],
                "wq": wq_h,
                "wk": wk_h,
                "wv": wv_h,
                "bqr": bq_h,
                "bkr": bk_h,
            }
        )

    res = run_bass_kernel_spmd(nc, in_maps, core_ids=list(range(NCORES)))
    out = np.stack([res.results[i]["y"] for i in range(NCORES)])
    # (NCORES, BPC, 2, P, N) -> (B, C, N) -> (B, C, H, W)
    out = out.reshape(B, C, N)
    return np.ascontiguousarray(out.reshape(B, C, H, W))


if __name__ == "__main__":
    t0 = time.time()
    nc = _build_nc()
    print(f"build ok: {time.time() - t0:.1f}s")


# revision 49
# speedup vs baseline: 1.1642x; 1.1642x over previous
"""Trainium2 Bass kernel for AdditiveAttention (per-batch bmm attention).

Per batch element b (x: (C, N) with C=256, KC=32, N=48*48=2304):
    q = Wq @ x + bq            (KC, N)
    k = Wk @ x + bk            (KC, N)
    v = Wv @ x + bv            (C, N)
    s = (q^T k) / sqrt(KC)     (N, N)
    a = softmax(s, axis=-1)
    out = v @ a^T              (C, N)
    y = gamma * out + x

Distribution: data-parallel over batch B=16 across 8 cores (2 per core);
the small channel-mixing weights are replicated.

Device strategy (v2 — fp8 DoubleRow + PE row-tiling):
  - x is shipped as fp8 "cc-pairs" [128, 2, N] so every producer matmul can
    use fp8 DoubleRow (contraction 256 = 2 k-tiles of 128 at 0.5 cyc/row).
  - q, k are produced REPLICATED 4x across partition quadrants (the Wq/Wk
    stationary is pre-tiled on the host), so the K=32 scores matmuls can be
    issued to distinct 32-row PE bands via tile_position and run
    concurrently (the 128x128 array is 16 independent 32x32 sub-arrays).
  - scoresT[j, i] chunks go through PSUM in 2-j-block groups; exp runs as
    1024-wide activations writing fp8 directly in the DoubleRow pair layout
    [128, 2, 512] that the out-matmul consumes.
  - out is computed in [c, i] layout: out[c,i] = sum_j v[c,j] e[j,i] via
    DoubleRow matmuls with stationary vt-pairs (fp8), slab-major so the
    PSUM accumulates over all 9 j-pairs per 512-i-chunk.
  - The softmax denominator D[i] = sum_j e[j,i] comes from an extra
    all-ones fp8 stationary slab -> D lands broadcast on all 128
    partitions; normalization is then a plain tensor_tensor multiply by
    reciprocal(D), and the residual (x + gamma*bv, folded on host) is a
    second tensor_tensor add.
  - gamma is folded into Wv on the host; gamma=0 stays exact because
    y = 0*rd + xfb = x in f32.
"""

import math
import time
from contextlib import ExitStack

import numpy as np

import concourse.bass as bass
import concourse.bacc as bacc
import concourse.mybir as mybir
import concourse.tile as tile
from concourse.bass_utils import run_bass_kernel_spmd

B, C, KC, H, W = 16, 256, 32, 48, 48
N = H * W            # 2304
NCORES = 8
BPC = B // NCORES    # batch elements per core = 2
P = 128
NB = N // P          # 18 j-blocks
NJP = NB // 2        # 9 j-pairs
CHUNKS = [(0, 512), (512, 512), (1024, 512), (1536, 512), (2048, 256)]

F32 = mybir.dt.float32
BF16 = mybir.dt.bfloat16
F8 = mybir.dt.float8e4
U8 = mybir.dt.uint8
DR = mybir.MatmulPerfMode.DoubleRow
EXP = mybir.ActivationFunctionType.Exp
R32 = 1.0 / math.sqrt(KC)
EXP_SHIFT = -3.0          # e = exp(s/sqrt(KC) - 3); cancels in softmax, keeps e < fp8e4m3 max
# Schraudolph exp constants for fp8e4m3 bit patterns (bias 7, 3 mantissa):
SCH_A = (8.0 / math.log(2.0)) * R32
SCH_B = 7 * 8 + EXP_SHIFT * (8.0 / math.log(2.0)) - 0.463

# exp engine split by group parity: even groups (psum buffer chain A) stay on
# ScalarE, odd groups go to DVE via Schraudolph -> the two serial exp chains
# advance concurrently on two engines.
DVE_EXP_G = frozenset({1, 3, 5, 7})


class _Builder:
    def __init__(self, dve_exp_g=DVE_EXP_G):
        nc = bacc.Bacc()
        self.nc = nc
        self.dve_exp_g = dve_exp_g
        self.xp = nc.dram_tensor("xp", [BPC, P, 2, N], F8, kind="ExternalInput")
        self.xfb = nc.dram_tensor("xfb", [BPC, 2, P, N], F32, kind="ExternalInput")
        self.wq = nc.dram_tensor("wq", [P, 2, P], F8, kind="ExternalInput")
        self.wk = nc.dram_tensor("wk", [P, 2, P], F8, kind="ExternalInput")
        self.wv = nc.dram_tensor("wv", [P, 2, C], F8, kind="ExternalInput")
        self.bqr = nc.dram_tensor("bqr", [P, 1], F32, kind="ExternalInput")
        self.bkr = nc.dram_tensor("bkr", [P, 1], F32, kind="ExternalInput")
        self.y = nc.dram_tensor("y", [BPC, 2, P, N], F32, kind="ExternalOutput")

    def build(self):
        nc = self.nc
        with tile.TileContext(nc) as tc, ExitStack() as ctx:
            self.tc = tc
            const = ctx.enter_context(tc.tile_pool(name="const", bufs=1))
            # PSUM: 4 (scores) + 3 (out D/c0/c1) + 1 (aux) banks
            self.ps_s = ctx.enter_context(tc.tile_pool(name="ps_s", bufs=2, space="PSUM"))
            self.ps_o = ctx.enter_context(tc.tile_pool(name="ps_o", bufs=2, space="PSUM"))
            self.ps_a = ctx.enter_context(tc.tile_pool(name="ps_a", bufs=1, space="PSUM"))
            self.xppool = ctx.enter_context(tc.tile_pool(name="xppool", bufs=2))
            self.qkpool = ctx.enter_context(tc.tile_pool(name="qkpool", bufs=4))
            self.vtpool = ctx.enter_context(tc.tile_pool(name="vtpool", bufs=2 * NJP))
            self.epool = ctx.enter_context(tc.tile_pool(name="epool", bufs=20))
            self.xfpool = ctx.enter_context(tc.tile_pool(name="xfpool", bufs=8))
            self.ypool = ctx.enter_context(tc.tile_pool(name="ypool", bufs=4))
            self.tmppool = ctx.enter_context(tc.tile_pool(name="tmppool", bufs=4))
            self.rdpool = ctx.enter_context(tc.tile_pool(name="rdpool", bufs=3))

            # ---- consts ----
            warm_act = const.tile([1, 2], F32)
            nc.vector.memset(warm_act, 0.0)
            nc.scalar.activation(out=warm_act, in_=warm_act, func=EXP)
            self.ones_sb = const.tile([P, 2, P], F8)
            nc.gpsimd.memset(self.ones_sb, 1.0)
            self.eshift = const.tile([P, 1], F32)
            nc.vector.memset(self.eshift, EXP_SHIFT)
            self.wq_sb = const.tile([P, 2, P], F8)
            nc.sync.dma_start(out=self.wq_sb, in_=self.wq[:])
            self.wk_sb = const.tile([P, 2, P], F8)
            nc.sync.dma_start(out=self.wk_sb, in_=self.wk[:])
            self.wv_sb = const.tile([P, 2, C], F8)
            nc.sync.dma_start(out=self.wv_sb, in_=self.wv[:])
            self.bq_sb = const.tile([P, 1], F32)
            nc.sync.dma_start(out=self.bq_sb, in_=self.bqr[:])
            self.bk_sb = const.tile([P, 1], F32)
            nc.sync.dma_start(out=self.bk_sb, in_=self.bkr[:])

            # two aux PSUM banks; producers fill 256-halves, copies drain 512.
            # q/vt use the low bank (DVE copies), k the high bank (ScalarE).
            self.aux = self.ps_a.tile([P, 1024], F32, tag="a", name="aux_ps")
            self.exp_ctr = 0

            # ---- HAM warmup: dense dummy matmuls (via ps_o so qk can start) ----
            wtile = const.tile([P, 256], BF16)
            nc.vector.memset(wtile, 0.0)
            for wi in range(21):
                ps = self.ps_o.tile([P, 512], F32, tag="o", name="warm_ps")
                nc.tensor.matmul(
                    ps[:, 0:256], lhsT=wtile[:, 0:P], rhs=wtile[:, 0:256],
                    start=True, stop=True,
                )

            # ---- per-core state ----
            self.xs = {}    # b -> xp tile [P, 2, N] F8
            self.q = {}     # b -> q tile [P, N] BF16 (4x replicated on partitions)
            self.k = {}
            self.vtp = {}   # (b, jp) -> [P, 2, C] F8
            self.es = {}    # (b, t, g) -> e tile [P, 2, 512] F8
            self.xf = {}    # (b, t, s) -> xf tile
            self.rd = {}    # (b, t) -> reciprocal-D tile

            self.emit_x_load(0)
            self.emit_x_load(1)
            self.emit_qk(0)

            # startup: scores(b0,t0) interleaved with vt(b0)
            for g in range(NJP):
                self.emit_scores_exp(0, 0, g)
                self.emit_vt(0, 2 * g)
                self.emit_vt(0, 2 * g + 1)

            seq = [(b, t) for b in range(BPC) for t in range(len(CHUNKS))]
            for idx, (b, t) in enumerate(seq):
                nxt = seq[idx + 1] if idx + 1 < len(seq) else None
                self.emit_chunk(b, t, nxt, idx)
                # inject b1 prologue early
                if idx == 0:
                    self.emit_qk(1)
                if idx == 1:
                    for j in range(NB):
                        self.emit_vt(1, j)

        nc.finalize()
        return nc

    # ---------- phases ----------

    def emit_x_load(self, b):
        # split across two queues so neither blocks the (tiny) const loads
        nc = self.nc
        self.xs[b] = self.xppool.tile([P, 2, N], F8, tag="xp", name="x_sb")
        half = N // 2
        nc.gpsimd.dma_start(
            out=self.xs[b][:, :, 0:half], in_=self.xp[b][:, :, 0:half]
        )
        nc.scalar.dma_start(
            out=self.xs[b][:, :, half:N], in_=self.xp[b][:, :, half:N]
        )

    def emit_qk(self, b, borrow=False):
        """q/k in 512-col steps (two 256 DR matmuls + one wide biased copy).

        k goes through a borrowed ps_o bank with copies on ScalarE, q through
        the aux bank with copies on DVE — two independent chains, so the
        startup latency before scores can begin is roughly halved.
        """
        nc = self.nc
        self.q[b] = self.qkpool.tile([P, N], BF16, tag="q", name="q_sb")
        self.k[b] = self.qkpool.tile([P, N], BF16, tag="k", name="k_sb")
        steps = [(512 * h, 512) for h in range(4)] + [(2048, 256)]
        for i0, w in steps:
            for ci, (w_sb, b_sb, dst) in enumerate(
                (
                    (self.wk_sb, self.bk_sb, self.k[b]),
                    (self.wq_sb, self.bq_sb, self.q[b]),
                )
            ):
                ps = self.aux[:, 512 * ci : 512 * ci + 512]
                nc.tensor.matmul(
                    ps[:, 0:w],
                    lhsT=w_sb,
                    rhs=self.xs[b][:, :, i0 : i0 + w],
                    perf_mode=DR,
                    start=True,
                    stop=True,
                )
                if ci == 0:
                    nc.scalar.activation(
                        out=dst[:, i0 : i0 + w], in_=ps[:, 0:w],
                        func=mybir.ActivationFunctionType.Identity, bias=b_sb,
                    )
                else:
                    nc.vector.tensor_scalar_add(dst[:, i0 : i0 + w], ps[:, 0:w], b_sb)

    def emit_vt(self, b, j):
        """vt[j] = (x_j^T) @ WvT in (j-part, c) layout, stored into fp8 pair
        tiles; the psum->fp8 copy drains a full j-pair (512 wide) at once."""
        nc = self.nc
        jp, par = divmod(j, 2)
        if par == 0:
            self.vtp[(b, jp)] = self.vtpool.tile([P, 2, C], F8, tag="vtp", name="vtp")
        ps = self.aux[:, 512 * (jp % 2) + 256 * par : 512 * (jp % 2) + 256 * (par + 1)]
        nc.tensor.matmul(
            ps,
            lhsT=self.xs[b][:, :, j * P : (j + 1) * P],
            rhs=self.wv_sb,
            perf_mode=DR,
            start=True,
            stop=True,
        )
        if par == 1:
            lo = 512 * (jp % 2)
            nc.scalar.activation(
                out=self.vtp[(b, jp)][:, :, :], in_=self.aux[:, lo : lo + 512],
                func=mybir.ActivationFunctionType.Copy,
            )

    def emit_scores_exp(self, b, t, g):
        """One 2-j-block scores group + its 1024-wide exp -> fp8 e pair tile."""
        nc = self.nc
        i0, w = CHUNKS[t]
        ps = self.ps_s.tile([P, 2, 512], F32, tag="s", name="s_ps")
        band = (g % 2) * 2  # alternate row-band pairs (0,1) / (2,3)
        for r in range(2):
            j = 2 * g + r
            rb = band + r
            nc.tensor.matmul(
                ps[:, r, 0:w],
                lhsT=self.k[b][32 * rb : 32 * (rb + 1), j * P : (j + 1) * P],
                rhs=self.q[b][32 * rb : 32 * (rb + 1), i0 : i0 + w],
                tile_position=(32 * rb, 0),
                start=True,
                stop=True,
            )
        e = self.epool.tile([P, 2, 512], F8, tag="e", name="e_sb")
        self.exp_ctr += 1
        if self.dve_exp_g and (self.exp_ctr % 2 == 0):
            # Schraudolph exp: fp8 bits = round(s*A + B), saturating convert
            # through uint8 clamps the deep-negative tail to +0.
            nc.vector.tensor_scalar(
                out=e[:, :, 0:w].bitcast(U8),
                in0=ps[:, :, 0:w],
                scalar1=SCH_A,
                scalar2=SCH_B,
                op0=mybir.AluOpType.mult,
                op1=mybir.AluOpType.add,
            )
        else:
            nc.scalar.activation(
                out=e[:, :, 0:w], in_=ps[:, :, 0:w], func=EXP,
                scale=R32, bias=self.eshift,
            )
        self.es[(b, t, g)] = e

    def emit_out_mm(self, b, t, mm_i, slabs):
        """One out-burst step: DR matmul mm_i (slab-major) + epilogue hooks.

        slabs holds [D, c0] from the chunk start; c1's psum tile is allocated
        lazily after the D reciprocal so it can reuse D's bank (ps_o bufs=2).
        """
        nc = self.nc
        i0, w = CHUNKS[t]
        kind, jp = divmod(mm_i, NJP)
        e = self.es[(b, t, jp)]
        lhsT = (
            self.ones_sb
            if kind == 0
            else self.vtp[(b, jp)][:, :, (kind - 1) * P : kind * P]
        )
        nc.tensor.matmul(
            slabs[kind][:, 0:w],
            lhsT=lhsT,
            rhs=e[:, :, 0:w],
            perf_mode=DR,
            start=(jp == 0),
            stop=(jp == NJP - 1),
        )
        mm_i += 1
        if mm_i == 9:
            # D slab complete -> reciprocal (fast approx; D is O(1)..O(500))
            rd = self.rdpool.tile([P, 512], F32, tag="rd", name="rd_sb")
            nc.vector.reciprocal_approx_fast(rd[:, 0:w], slabs[0][:, 0:w])
            self.rd[(b, t)] = rd
            slabs.append(self.ps_o.tile([P, 512], F32, tag="o", name="o_ps"))
        elif mm_i in (18, 27):
            s = mm_i // 9 - 2  # 0 or 1
            tmp = self.tmppool.tile([P, 512], F32, tag="tmp", name="tmp_sb")
            nc.vector.tensor_mul(
                tmp[:, 0:w], slabs[s + 1][:, 0:w], self.rd[(b, t)][:, 0:w]
            )
            yt = self.ypool.tile([P, 512], F32, tag="y", name="y_sb")
            add_eng = nc.vector if (b, t) == (BPC - 1, len(CHUNKS) - 1) else nc.gpsimd
            add_eng.tensor_add(
                yt[:, 0:w], tmp[:, 0:w], self.xf[(b, t, s)][:, 0:w]
            )
            nc.sync.dma_start(out=self.y[b, s, :, i0 : i0 + w], in_=yt[:, 0:w])

    def emit_chunk(self, b, t, nxt, idx):
        """Out-burst + epilogue for (b, t), interleaved with scores for nxt.

        Scores groups are emitted in adjacent pairs (bands 0,1 then 2,3) so
        up to 4 small-K matmuls stream concurrently in distinct PE row
        bands; out-burst DR matmuls fill the gaps while ScalarE works
        through the exps.
        """
        nc = self.nc
        i0, w = CHUNKS[t]

        # prefetch xf one chunk ahead so the epilogue add never waits the DMA
        for tb, tt in ((b, t), nxt or (None, None)):
            if tb is None or (tb, tt, 0) in self.xf:
                continue
            ii0, ww = CHUNKS[tt]
            for s in range(2):
                xf = self.xfpool.tile([P, 512], F32, tag="xf", name="xf_sb")
                nc.sync.dma_start(
                    out=xf[:, 0:ww], in_=self.xfb[tb, s, :, ii0 : ii0 + ww]
                )
                self.xf[(tb, tt, s)] = xf

        slabs = [
            self.ps_o.tile([P, 512], F32, tag="o", name="o_ps") for _ in range(2)
        ]

        # interleave: scores pair-bursts of nxt with out-burst of this chunk
        mm_i = 0
        for h in range(5):
            if nxt is not None:
                for g in (2 * h, 2 * h + 1):
                    if g < NJP:
                        self.emit_scores_exp(nxt[0], nxt[1], g)
            n_out = 6 if h < 4 else 3
            for _ in range(n_out):
                self.emit_out_mm(b, t, mm_i, slabs)
                mm_i += 1


def _build_nc(dve_exp_g=DVE_EXP_G):
    return _Builder(dve_exp_g).build()


_CACHE = {}


def kernel(x, Wq, bq, Wk, bk, Wv, bv, gamma):
    x = np.asarray(x, dtype=np.float32)
    Wq = np.asarray(Wq, dtype=np.float32)
    bq = np.asarray(bq, dtype=np.float32)
    Wk = np.asarray(Wk, dtype=np.float32)
    bk = np.asarray(bk, dtype=np.float32)
    Wv = np.asarray(Wv, dtype=np.float32)
    bv = np.asarray(bv, dtype=np.float32)
    gamma = np.asarray(gamma, dtype=np.float32)
    g = float(gamma[0])
    f8 = mybir.dt.np(F8)

    xfull = x.reshape(B, C, N)

    # x cc-pairs: [B, P, 2, N]
    xpair = np.ascontiguousarray(xfull.reshape(B, 2, P, N).transpose(0, 2, 1, 3))
    # residual + gamma*bv folded: [B, 2, P, N]
    xf = xfull.reshape(B, 2, P, N) + (g * bv).reshape(1, 2, P, 1)
    xf = np.ascontiguousarray(xf, dtype=np.float32)

    def rep_pairs(wT):  # (C, K) -> [P, 2, 4*K] replicated along quadrants
        a = np.ascontiguousarray(wT.reshape(2, P, wT.shape[1]).transpose(1, 0, 2))
        return np.tile(a, (1, 1, 4))

    wq_h = rep_pairs(Wq.T).astype(f8)                       # (P, 2, 128)
    wk_h = rep_pairs(Wk.T).astype(f8)
    wv_h = np.ascontiguousarray(
        (Wv * g).T.reshape(2, P, C).transpose(1, 0, 2)
    ).astype(f8)                                            # (P, 2, 256)
    bq_h = np.ascontiguousarray(np.tile(bq, 4).reshape(P, 1), dtype=np.float32)
    bk_h = np.ascontiguousarray(np.tile(bk, 4).reshape(P, 1), dtype=np.float32)

    if "nc" not in _CACHE:
        _CACHE["nc"] = _build_nc()
    nc = _CACHE["nc"]

    in_maps = []
    for core in range(NCORES):
        bsl = slice(core * BPC, (core + 1) * BPC)
        in_maps.append(
            {
                "xp": xpair[bsl].astype(f8),
                "xfb": xf[bsl],
                "wq": wq_h,
                "wk": wk_h,
                "wv": wv_h,
                "bqr": bq_h,
                "bkr": bk_h,
            }
        )

    res = run_bass_kernel_spmd(nc, in_maps, core_ids=list(range(NCORES)))
    out = np.stack([res.results[i]["y"] for i in range(NCORES)])
    # (NCORES, BPC, 2, P, N) -> (B, C, N) -> (B, C, H, W)
    out = out.reshape(B, C, N)
    return np.ascontiguousarray(out.reshape(B, C, H, W))


if __name__ == "__main__":
    t0 = time.time()
    nc = _build_nc()
    print(f"build ok: {time.time() - t0:.1f}s")


# revision 54
# speedup vs baseline: 1.2002x; 1.0309x over previous
"""Trainium2 Bass kernel for AdditiveAttention (per-batch bmm attention).

Per batch element b (x: (C, N) with C=256, KC=32, N=48*48=2304):
    q = Wq @ x + bq            (KC, N)
    k = Wk @ x + bk            (KC, N)
    v = Wv @ x + bv            (C, N)
    s = (q^T k) / sqrt(KC)     (N, N)
    a = softmax(s, axis=-1)
    out = v @ a^T              (C, N)
    y = gamma * out + x

Distribution: data-parallel over batch B=16 across 8 cores (2 per core);
the small channel-mixing weights are replicated.

Device strategy (v2 — fp8 DoubleRow + PE row-tiling):
  - x is shipped as fp8 "cc-pairs" [128, 2, N] so every producer matmul can
    use fp8 DoubleRow (contraction 256 = 2 k-tiles of 128 at 0.5 cyc/row).
  - q, k are produced REPLICATED 4x across partition quadrants (the Wq/Wk
    stationary is pre-tiled on the host), so the K=32 scores matmuls can be
    issued to distinct 32-row PE bands via tile_position and run
    concurrently (the 128x128 array is 16 independent 32x32 sub-arrays).
  - scoresT[j, i] chunks go through PSUM in 2-j-block groups; exp runs as
    1024-wide activations writing fp8 directly in the DoubleRow pair layout
    [128, 2, 512] that the out-matmul consumes.
  - out is computed in [c, i] layout: out[c,i] = sum_j v[c,j] e[j,i] via
    DoubleRow matmuls with stationary vt-pairs (fp8), slab-major so the
    PSUM accumulates over all 9 j-pairs per 512-i-chunk.
  - The softmax denominator D[i] = sum_j e[j,i] comes from an extra
    all-ones fp8 stationary slab -> D lands broadcast on all 128
    partitions; normalization is then a plain tensor_tensor multiply by
    reciprocal(D), and the residual (x + gamma*bv, folded on host) is a
    second tensor_tensor add.
  - gamma is folded into Wv on the host; gamma=0 stays exact because
    y = 0*rd + xfb = x in f32.
"""

import math
import time
from contextlib import ExitStack

import numpy as np

import concourse.bass as bass
import concourse.bacc as bacc
import concourse.mybir as mybir
import concourse.tile as tile
from concourse.bass_utils import run_bass_kernel_spmd

B, C, KC, H, W = 16, 256, 32, 48, 48
N = H * W            # 2304
NCORES = 8
BPC = B // NCORES    # batch elements per core = 2
P = 128
NB = N // P          # 18 j-blocks
NJP = NB // 2        # 9 j-pairs
CHUNKS = [(0, 512), (512, 512), (1024, 512), (1536, 512), (2048, 256)]

F32 = mybir.dt.float32
BF16 = mybir.dt.bfloat16
F8 = mybir.dt.float8e4
U8 = mybir.dt.uint8
DR = mybir.MatmulPerfMode.DoubleRow
EXP = mybir.ActivationFunctionType.Exp
R32 = 1.0 / math.sqrt(KC)
EXP_SHIFT = -3.0          # e = exp(s/sqrt(KC) - 3); cancels in softmax, keeps e < fp8e4m3 max
# Schraudolph exp constants for fp8e4m3 bit patterns (bias 7, 3 mantissa):
SCH_A = (8.0 / math.log(2.0)) * R32
SCH_B = 7 * 8 + EXP_SHIFT * (8.0 / math.log(2.0)) - 0.463

# exp engine split by group parity: even groups (psum buffer chain A) stay on
# ScalarE, odd groups go to DVE via Schraudolph -> the two serial exp chains
# advance concurrently on two engines.
DVE_EXP_G = frozenset({1, 3, 5, 7})


class _Builder:
    def __init__(self, dve_exp_g=DVE_EXP_G):
        nc = bacc.Bacc()
        self.nc = nc
        self.dve_exp_g = dve_exp_g
        self.xp = nc.dram_tensor("xp", [BPC, P, 2, N], F8, kind="ExternalInput")
        self.xfb = nc.dram_tensor("xfb", [BPC, 2, P, N], F32, kind="ExternalInput")
        self.wq = nc.dram_tensor("wq", [P, 2, P], F8, kind="ExternalInput")
        self.wk = nc.dram_tensor("wk", [P, 2, P], F8, kind="ExternalInput")
        self.wv = nc.dram_tensor("wv", [P, 2, C], F8, kind="ExternalInput")
        self.bqr = nc.dram_tensor("bqr", [P, 1], F32, kind="ExternalInput")
        self.bkr = nc.dram_tensor("bkr", [P, 1], F32, kind="ExternalInput")
        self.y = nc.dram_tensor("y", [BPC, 2, P, N], F32, kind="ExternalOutput")

    def build(self):
        nc = self.nc
        with tile.TileContext(nc) as tc, ExitStack() as ctx:
            self.tc = tc
            const = ctx.enter_context(tc.tile_pool(name="const", bufs=1))
            # PSUM: 4 (scores) + 3 (out D/c0/c1) + 1 (aux) banks
            self.ps_s = ctx.enter_context(tc.tile_pool(name="ps_s", bufs=2, space="PSUM"))
            self.ps_o = ctx.enter_context(tc.tile_pool(name="ps_o", bufs=3, space="PSUM"))
            self.ps_a = ctx.enter_context(tc.tile_pool(name="ps_a", bufs=1, space="PSUM"))
            self.xppool = ctx.enter_context(tc.tile_pool(name="xppool", bufs=2))
            self.qkpool = ctx.enter_context(tc.tile_pool(name="qkpool", bufs=4))
            self.vtpool = ctx.enter_context(tc.tile_pool(name="vtpool", bufs=2 * NJP))
            self.epool = ctx.enter_context(tc.tile_pool(name="epool", bufs=20))
            self.xfpool = ctx.enter_context(tc.tile_pool(name="xfpool", bufs=8))
            self.ypool = ctx.enter_context(tc.tile_pool(name="ypool", bufs=4))
            self.tmppool = ctx.enter_context(tc.tile_pool(name="tmppool", bufs=4))
            self.rdpool = ctx.enter_context(tc.tile_pool(name="rdpool", bufs=3))

            # ---- consts ----
            warm_act = const.tile([1, 2], F32)
            nc.vector.memset(warm_act, 0.0)
            nc.scalar.activation(out=warm_act, in_=warm_act, func=EXP)
            self.ones_sb = const.tile([P, 2, P], F8)
            nc.gpsimd.memset(self.ones_sb, 1.0)
            self.eshift = const.tile([P, 1], F32)
            nc.vector.memset(self.eshift, EXP_SHIFT)
            self.wq_sb = const.tile([P, 2, P], F8)
            nc.sync.dma_start(out=self.wq_sb, in_=self.wq[:])
            self.wk_sb = const.tile([P, 2, P], F8)
            nc.sync.dma_start(out=self.wk_sb, in_=self.wk[:])
            self.wv_sb = const.tile([P, 2, C], F8)
            nc.sync.dma_start(out=self.wv_sb, in_=self.wv[:])
            self.bq_sb = const.tile([P, 1], F32)
            nc.sync.dma_start(out=self.bq_sb, in_=self.bqr[:])
            self.bk_sb = const.tile([P, 1], F32)
            nc.sync.dma_start(out=self.bk_sb, in_=self.bkr[:])

            # one aux PSUM bank shared by the q/k/vt producer round-trips
            self.aux = self.ps_a.tile([P, 512], F32, tag="a", name="aux_ps")
            self.exp_ctr = 0

            # ---- HAM warmup: dense dummy matmuls (via ps_o so qk can start) ----
            wtile = const.tile([P, 256], BF16)
            nc.vector.memset(wtile, 0.0)
            for wi in range(21):
                ps = self.ps_o.tile([P, 512], F32, tag="o", name="warm_ps")
                nc.tensor.matmul(
                    ps[:, 0:256], lhsT=wtile[:, 0:P], rhs=wtile[:, 0:256],
                    start=True, stop=True,
                )

            # ---- per-core state ----
            self.xs = {}    # b -> xp tile [P, 2, N] F8
            self.q = {}     # b -> q tile [P, N] BF16 (4x replicated on partitions)
            self.k = {}
            self.vtp = {}   # (b, jp) -> [P, 2, C] F8
            self.es = {}    # (b, t, g) -> e tile [P, 2, 512] F8
            self.xf = {}    # (b, t, s) -> xf tile
            self.rd = {}    # (b, t) -> reciprocal-D tile

            self.emit_x_load(0)
            self.emit_x_load(1)
            self.emit_qk(0)

            # startup: scores(b0,t0) interleaved with vt(b0)
            for g in range(NJP):
                self.emit_scores_exp(0, 0, g)
                self.emit_vt(0, 2 * g)
                self.emit_vt(0, 2 * g + 1)

            seq = [(b, t) for b in range(BPC) for t in range(len(CHUNKS))]
            for idx, (b, t) in enumerate(seq):
                nxt = seq[idx + 1] if idx + 1 < len(seq) else None
                self.emit_chunk(b, t, nxt, idx)
                # inject b1 prologue early
                if idx == 0:
                    self.emit_qk(1)
                if idx == 1:
                    for j in range(NB):
                        self.emit_vt(1, j)

        nc.finalize()
        return nc

    # ---------- phases ----------

    def emit_x_load(self, b):
        # split across two queues so neither blocks the (tiny) const loads
        nc = self.nc
        self.xs[b] = self.xppool.tile([P, 2, N], F8, tag="xp", name="x_sb")
        half = N // 2
        nc.gpsimd.dma_start(
            out=self.xs[b][:, :, 0:half], in_=self.xp[b][:, :, 0:half]
        )
        nc.scalar.dma_start(
            out=self.xs[b][:, :, half:N], in_=self.xp[b][:, :, half:N]
        )

    def emit_qk(self, b, borrow=False):
        """q/k in 512-col steps (two 256 DR matmuls + one wide biased copy).

        k goes through a borrowed ps_o bank with copies on ScalarE, q through
        the aux bank with copies on DVE — two independent chains, so the
        startup latency before scores can begin is roughly halved.
        """
        nc = self.nc
        self.q[b] = self.qkpool.tile([P, N], BF16, tag="q", name="q_sb")
        self.k[b] = self.qkpool.tile([P, N], BF16, tag="k", name="k_sb")
        steps = [(512 * h, 512) for h in range(4)] + [(2048, 256)]
        for i0, w in steps:
            for ci, (w_sb, b_sb, dst) in enumerate(
                (
                    (self.wk_sb, self.bk_sb, self.k[b]),
                    (self.wq_sb, self.bq_sb, self.q[b]),
                )
            ):
                ps = self.aux[:, 0:512]
                nc.tensor.matmul(
                    ps[:, 0:w],
                    lhsT=w_sb,
                    rhs=self.xs[b][:, :, i0 : i0 + w],
                    perf_mode=DR,
                    start=True,
                    stop=True,
                )
                if ci == 0:
                    nc.scalar.activation(
                        out=dst[:, i0 : i0 + w], in_=ps[:, 0:w],
                        func=mybir.ActivationFunctionType.Identity, bias=b_sb,
                    )
                else:
                    nc.vector.tensor_scalar_add(dst[:, i0 : i0 + w], ps[:, 0:w], b_sb)

    def emit_vt(self, b, j):
        """vt[j] = (x_j^T) @ WvT in (j-part, c) layout, stored into fp8 pair
        tiles; the psum->fp8 copy drains a full j-pair (512 wide) at once."""
        nc = self.nc
        jp, par = divmod(j, 2)
        if par == 0:
            self.vtp[(b, jp)] = self.vtpool.tile([P, 2, C], F8, tag="vtp", name="vtp")
        ps = self.aux[:, 256 * par : 256 * (par + 1)]
        nc.tensor.matmul(
            ps,
            lhsT=self.xs[b][:, :, j * P : (j + 1) * P],
            rhs=self.wv_sb,
            perf_mode=DR,
            start=True,
            stop=True,
        )
        if par == 1:
            nc.scalar.activation(
                out=self.vtp[(b, jp)][:, :, :], in_=self.aux[:, 0:512],
                func=mybir.ActivationFunctionType.Copy,
            )

    def emit_scores_exp(self, b, t, g):
        """One 2-j-block scores group + its 1024-wide exp -> fp8 e pair tile."""
        nc = self.nc
        i0, w = CHUNKS[t]
        ps = self.ps_s.tile([P, 2, 512], F32, tag="s", name="s_ps")
        band = (g % 2) * 2  # alternate row-band pairs (0,1) / (2,3)
        for r in range(2):
            j = 2 * g + r
            rb = band + r
            nc.tensor.matmul(
                ps[:, r, 0:w],
                lhsT=self.k[b][32 * rb : 32 * (rb + 1), j * P : (j + 1) * P],
                rhs=self.q[b][32 * rb : 32 * (rb + 1), i0 : i0 + w],
                tile_position=(32 * rb, 0),
                start=True,
                stop=True,
            )
        e = self.epool.tile([P, 2, 512], F8, tag="e", name="e_sb")
        self.exp_ctr += 1
        if self.dve_exp_g and (self.exp_ctr % 2 == 0):
            # Schraudolph exp: fp8 bits = round(s*A + B), saturating convert
            # through uint8 clamps the deep-negative tail to +0.
            nc.vector.tensor_scalar(
                out=e[:, :, 0:w].bitcast(U8),
                in0=ps[:, :, 0:w],
                scalar1=SCH_A,
                scalar2=SCH_B,
                op0=mybir.AluOpType.mult,
                op1=mybir.AluOpType.add,
            )
        else:
            nc.scalar.activation(
                out=e[:, :, 0:w], in_=ps[:, :, 0:w], func=EXP,
                scale=R32, bias=self.eshift,
            )
        self.es[(b, t, g)] = e

    def emit_out_mm(self, b, t, mm_i, slabs):
        """One out-burst step: DR matmul mm_i (slab-major) + epilogue hooks.

        slabs holds [D, c0] from the chunk start; c1's psum tile is allocated
        lazily after the D reciprocal so it can reuse D's bank (ps_o bufs=2).
        """
        nc = self.nc
        i0, w = CHUNKS[t]
        kind, jp = divmod(mm_i, NJP)
        e = self.es[(b, t, jp)]
        lhsT = (
            self.ones_sb
            if kind == 0
            else self.vtp[(b, jp)][:, :, (kind - 1) * P : kind * P]
        )
        nc.tensor.matmul(
            slabs[kind][:, 0:w],
            lhsT=lhsT,
            rhs=e[:, :, 0:w],
            perf_mode=DR,
            start=(jp == 0),
            stop=(jp == NJP - 1),
        )
        mm_i += 1
        if mm_i == 9:
            # D slab complete -> reciprocal (fast approx; D is O(1)..O(500))
            rd = self.rdpool.tile([P, 512], F32, tag="rd", name="rd_sb")
            nc.vector.reciprocal_approx_fast(rd[:, 0:w], slabs[0][:, 0:w])
            self.rd[(b, t)] = rd
            slabs.append(self.ps_o.tile([P, 512], F32, tag="o", name="o_ps"))
        elif mm_i in (18, 27):
            s = mm_i // 9 - 2  # 0 or 1
            tmp = self.tmppool.tile([P, 512], F32, tag="tmp", name="tmp_sb")
            nc.vector.tensor_mul(
                tmp[:, 0:w], slabs[s + 1][:, 0:w], self.rd[(b, t)][:, 0:w]
            )
            yt = self.ypool.tile([P, 512], F32, tag="y", name="y_sb")
            add_eng = nc.vector if (b, t) == (BPC - 1, len(CHUNKS) - 1) else nc.gpsimd
            add_eng.tensor_add(
                yt[:, 0:w], tmp[:, 0:w], self.xf[(b, t, s)][:, 0:w]
            )
            nc.sync.dma_start(out=self.y[b, s, :, i0 : i0 + w], in_=yt[:, 0:w])

    def emit_chunk(self, b, t, nxt, idx):
        """Out-burst + epilogue for (b, t), interleaved with scores for nxt.

        Scores groups are emitted in adjacent pairs (bands 0,1 then 2,3) so
        up to 4 small-K matmuls stream concurrently in distinct PE row
        bands; out-burst DR matmuls fill the gaps while ScalarE works
        through the exps.
        """
        nc = self.nc
        i0, w = CHUNKS[t]

        # prefetch xf one chunk ahead so the epilogue add never waits the DMA
        for tb, tt in ((b, t), nxt or (None, None)):
            if tb is None or (tb, tt, 0) in self.xf:
                continue
            ii0, ww = CHUNKS[tt]
            for s in range(2):
                xf = self.xfpool.tile([P, 512], F32, tag="xf", name="xf_sb")
                nc.sync.dma_start(
                    out=xf[:, 0:ww], in_=self.xfb[tb, s, :, ii0 : ii0 + ww]
                )
                self.xf[(tb, tt, s)] = xf

        slabs = [
            self.ps_o.tile([P, 512], F32, tag="o", name="o_ps") for _ in range(2)
        ]

        # interleave: scores pair-bursts of nxt with out-burst of this chunk
        mm_i = 0
        for h in range(5):
            if nxt is not None:
                for g in (2 * h, 2 * h + 1):
                    if g < NJP:
                        self.emit_scores_exp(nxt[0], nxt[1], g)
            n_out = 6 if h < 4 else 3
            for _ in range(n_out):
                self.emit_out_mm(b, t, mm_i, slabs)
                mm_i += 1


def _build_nc(dve_exp_g=DVE_EXP_G):
    return _Builder(dve_exp_g).build()


_CACHE = {}


def kernel(x, Wq, bq, Wk, bk, Wv, bv, gamma):
    x = np.asarray(x, dtype=np.float32)
    Wq = np.asarray(Wq, dtype=np.float32)
    bq = np.asarray(bq, dtype=np.float32)
    Wk = np.asarray(Wk, dtype=np.float32)
    bk = np.asarray(bk, dtype=np.float32)
    Wv = np.asarray(Wv, dtype=np.float32)
    bv = np.asarray(bv, dtype=np.float32)
    gamma = np.asarray(gamma, dtype=np.float32)
    g = float(gamma[0])
    f8 = mybir.dt.np(F8)

    xfull = x.reshape(B, C, N)

    # x cc-pairs: [B, P, 2, N]
    xpair = np.ascontiguousarray(xfull.reshape(B, 2, P, N).transpose(0, 2, 1, 3))
    # residual + gamma*bv folded: [B, 2, P, N]
    xf = xfull.reshape(B, 2, P, N) + (g * bv).reshape(1, 2, P, 1)
    xf = np.ascontiguousarray(xf, dtype=np.float32)

    def rep_pairs(wT):  # (C, K) -> [P, 2, 4*K] replicated along quadrants
        a = np.ascontiguousarray(wT.reshape(2, P, wT.shape[1]).transpose(1, 0, 2))
        return np.tile(a, (1, 1, 4))

    wq_h = rep_pairs(Wq.T).astype(f8)                       # (P, 2, 128)
    wk_h = rep_pairs(Wk.T).astype(f8)
    wv_h = np.ascontiguousarray(
        (Wv * g).T.reshape(2, P, C).transpose(1, 0, 2)
    ).astype(f8)                                            # (P, 2, 256)
    bq_h = np.ascontiguousarray(np.tile(bq, 4).reshape(P, 1), dtype=np.float32)
    bk_h = np.ascontiguousarray(np.tile(bk, 4).reshape(P, 1), dtype=np.float32)

    if "nc" not in _CACHE:
        _CACHE["nc"] = _build_nc()
    nc = _CACHE["nc"]

    in_maps = []
    for core in range(NCORES):
        bsl = slice(core * BPC, (core + 1) * BPC)
        in_maps.append(
            {
                "xp": xpair[bsl].astype(f8),
                "xfb": xf[bsl],
                "wq": wq_h,
                "wk": wk_h,
                "wv": wv_h,
                "bqr": bq_h,
                "bkr": bk_h,
            }
        )

    res = run_bass_kernel_spmd(nc, in_maps, core_ids=list(range(NCORES)))
    out = np.stack([res.results[i]["y"] for i in range(NCORES)])
    # (NCORES, BPC, 2, P, N) -> (B, C, N) -> (B, C, H, W)
    out = out.reshape(B, C, N)
    return np.ascontiguousarray(out.reshape(B, C, H, W))


if __name__ == "__main__":
    t0 = time.time()
    nc = _build_nc()
    print(f"build ok: {time.time() - t0:.1f}s")
